# revision 11
# baseline (speedup 1.0000x reference)
"""HKSA block kernel for Trainium2: RoPE causal attention + block-diagonal LRU.

Shapes (hardcoded): B=2, T=1024, D=1024, NH=16, HD=64, M=16, H=64.

Sharding (8 NeuronCores):
  - Attention: core c handles batch b=c//4, head-group c%4 (4 heads). Each
    core emits a partial delta (o @ w_attn_out rows); host sums 4 partials
    per batch into the residual.
  - LRU: core c handles batch b=c//4 and 16 of the 64 LRU blocks. Each core
    emits a partial delta (h_out @ w_out_proj rows); host sums.

Inside a core the LRU scan runs on the Vector engine:
  state rows (unit u, i) live at partition 32a+16w+i (u = 8g+2a+w, 2 groups
  g of 8 units); per step a scalar_tensor_tensor computes the 16-wide
  matvec per state row via a zero-padded 32-wide gate window, and a 32x32
  StreamTranspose rebroadcasts the new state column into free-dim form.
"""

import math
import sys

if "/opt/trn_rl_repo" not in sys.path:
    sys.path.insert(0, "/opt/trn_rl_repo")

import numpy as np
import ml_dtypes

import concourse.bass as bass
import concourse.mybir as mybir
import concourse.tile as tile
from concourse import bacc
from concourse import bass_utils
from concourse.bass_interp import get_hw_module

B, T, D = 2, 1024, 1024
NH, HD = 16, 64
M = 16
H = D // M
EPS = 1e-5
ROPE_BASE = 10000.0

F32 = mybir.dt.float32
BF16 = mybir.dt.bfloat16
BF = ml_dtypes.bfloat16

MULT = mybir.AluOpType.mult
ADD = mybir.AluOpType.add
EXP = mybir.ActivationFunctionType.Exp

N_CORES = 8
HPC = H // 4            # 16 lru blocks per core
UPG = 8                 # units per scan group
TH = T // 2             # 512, T half
TTILE = 128             # scan gate tile

_cache = {}


# ---------------------------------------------------------------------------
# helpers
# ---------------------------------------------------------------------------

def _rmsnorm(x, w):
    ms = np.mean(x.astype(np.float64) ** 2, axis=-1, keepdims=True)
    return (x * (1.0 / np.sqrt(ms + EPS))).astype(np.float32) * w


def _unit_map():
    """partition p (0..127), group g -> (local unit u, state i)."""
    out = []
    for p in range(128):
        a, r = divmod(p, 32)
        w, i = divmod(r, 16)
        out.append((2 * a + w, i))
    return out  # u'' (0..7 within group), i


# ---------------------------------------------------------------------------
# LRU launch (part 2)
# ---------------------------------------------------------------------------

def build_lru_nc(finalize=True):
    """One SPMD program: per-core LRU block. Inputs are per-core shards."""
    nc = bacc.Bacc("TRN2", target_bir_lowering=False, debug=False,
                   enable_asserts=False, num_devices=N_CORES)
    h2_d = nc.dram_tensor("h2t", [128, 8, T], BF16, kind="ExternalInput").ap()
    wa_d = nc.dram_tensor("wa", [2, 17, 128, 8, 128], BF16, kind="ExternalInput").ap()
    wv_d = nc.dram_tensor("wv", [128, 8, 2, 128], BF16, kind="ExternalInput").ap()
    wo_d = nc.dram_tensor("wo", [128, 2, D], BF16, kind="ExternalInput").ap()
    mask_d = nc.dram_tensor("mask", [128, 2], F32, kind="ExternalInput").ap()
    out_d = nc.dram_tensor("delta", [128, 8, T], F32, kind="ExternalOutput").ap()

    with tile.TileContext(nc) as tc:
        with (
            tc.tile_pool(name="big", bufs=1) as big,
            tc.tile_pool(name="wstream", bufs=3) as wstream,
            tc.tile_pool(name="gp", bufs=2) as gpool,
            tc.tile_pool(name="outp", bufs=3) as outp,
            tc.tile_pool(name="psum", bufs=2, space="PSUM") as psum,
        ):
            h2_sb = big.tile([128, 8, T], BF16)
            wv_sb = big.tile([128, 8, 2, 128], BF16)
            wo_sb = big.tile([128, 2, D], BF16)
            mask_sb = big.tile([128, 2], F32)
            v_sb = big.tile([128, 2, T], F32)
            a0e = big.tile([128, 2, TH], F32)
            dinv = big.tile([128, 2, TH], F32)
            gA = big.tile([128, 2, TH, 16], BF16)
            a0v2 = big.tile([128, 2, T], F32)
            H2 = big.tile([128, 2, T], F32)
            Hb = big.tile([128, 2, T], BF16)
            tr2 = big.tile([128, 64], F32)
            rep2 = big.tile([128, 64], F32)
            scr = big.tile([128, 32], F32)

            nc.sync.dma_start(h2_sb[:], h2_d[:])
            nc.sync.dma_start(wv_sb[:], wv_d[:])
            nc.sync.dma_start(wo_sb[:], wo_d[:])
            nc.sync.dma_start(mask_sb[:], mask_d[:])
            nc.vector.memset(tr2[:], 0.0)

            # ---- v = h2 @ w_v  (feature-major, scan partition layout) ----
            for g in range(2):
                for th in range(2):
                    pv = psum.tile([128, TH], F32, name="pv")
                    for dc in range(8):
                        nc.tensor.matmul(
                            pv[:], wv_sb[:, dc, g], h2_sb[:, dc, bass.ts(th, TH)],
                            start=(dc == 0), stop=(dc == 7),
                        )
                    nc.scalar.copy(v_sb[:, g, bass.ts(th, TH)], pv[:])

            # ---- per T-half: gates -> softmax -> pad -> scan ----
            for th in range(2):
                for g in range(2):
                    for tau in range(17):
                        wa_sb = wstream.tile([128, 8, 128], BF16, name="wa_sb")
                        nc.sync.dma_start(wa_sb[:], wa_d[g, tau])
                        pg = psum.tile([128, TH], F32, name="pg")
                        for dc in range(8):
                            nc.tensor.matmul(
                                pg[:], wa_sb[:, dc], h2_sb[:, dc, bass.ts(th, TH)],
                                start=(dc == 0), stop=(dc == 7),
                            )
                        if tau == 0:
                            nc.scalar.activation(a0e[:, g], pg[:], EXP)
                        else:
                            nc.scalar.activation(gA[:, g, :, tau - 1], pg[:], EXP)
                        if tau == 0:
                            nc.vector.tensor_copy(dinv[:, g], a0e[:, g])
                        else:
                            nc.vector.tensor_add(
                                dinv[:, g], dinv[:, g], gA[:, g, :, tau - 1])
                    nc.vector.reciprocal(dinv[:, g], dinv[:, g])
                    # a0n = a0e * dinv ; a0v = a0n * v
                    nc.vector.tensor_mul(a0e[:, g], a0e[:, g], dinv[:, g])
                    nc.vector.tensor_tensor(
                        a0v2[:, g, bass.ts(th, TH)], a0e[:, g],
                        v_sb[:, g, bass.ts(th, TH)], MULT,
                    )
                    # gA *= dinv (broadcast over k)
                    nc.vector.tensor_tensor(
                        gA[:, g], gA[:, g],
                        dinv[:, g, :, None].to_broadcast([128, TH, 16]),
                        MULT,
                    )
                # pad + scan per 128-step tile
                for tt in range(TH // TTILE):
                    gp = []
                    for g in range(2):
                        gp_t = gpool.tile([128, TTILE, 32], BF16, tag=f"gp{g}",
                                          name=f"gp{g}")
                        for w in range(2):
                            nc.vector.tensor_scalar_mul(
                                gp_t[:, :, bass.ts(w, 16)],
                                gA[:, g, bass.ts(tt, TTILE)],
                                mask_sb[:, w:w + 1],
                            )
                        gp.append(gp_t)
                    for tl in range(TTILE):
                        t = th * TH + tt * TTILE + tl
                        nc.vector.scalar_tensor_tensor(
                            out=scr[:], in0=gp[0][:, tl], scalar=1.0,
                            in1=tr2[:, 0:32], op0=MULT, op1=MULT,
                            accum_out=H2[:, 0, t:t + 1],
                        )
                        nc.vector.scalar_tensor_tensor(
                            out=scr[:], in0=gp[1][:, tl], scalar=1.0,
                            in1=tr2[:, 32:64], op0=MULT, op1=MULT,
                            accum_out=H2[:, 1, t:t + 1],
                        )
                        nc.vector.tensor_tensor(
                            rep2[:].rearrange("p (g j) -> p g j", g=2),
                            H2[:, :, t].to_broadcast([128, 2, 32]),
                            a0v2[:, :, t].to_broadcast([128, 2, 32]),
                            ADD,
                        )
                        nc.vector.transpose(tr2[:], rep2[:])

            # ---- H = H2 + a0v (bf16) ; delta = wo.T @ H ----
            for g in range(2):
                nc.vector.tensor_tensor(
                    Hb[:, g], H2[:, g], a0v2[:, g], ADD,
                )
            for m in range(8):
                for th in range(2):
                    po = psum.tile([128, TH], F32, name="po")
                    for g in range(2):
                        nc.tensor.matmul(
                            po[:], wo_sb[:, g, bass.ts(m, 128)],
                            Hb[:, g, bass.ts(th, TH)],
                            start=(g == 0), stop=(g == 1),
                        )
                    ot = outp.tile([128, TH], F32, name="ot")
                    nc.scalar.copy(ot[:], po[:])
                    nc.sync.dma_start(out_d[:, m, bass.ts(th, TH)], ot[:])

    if finalize:
        nc.finalize()
        nc.m = get_hw_module(nc.m)
    return nc


def lru_shards(h2, w_v, w_a, w_out_proj):
    """Build per-core input maps for the LRU launch. h2: [B, T, D] f32."""
    umap = _unit_map()
    h2t = np.ascontiguousarray(h2.transpose(0, 2, 1))  # [B, D, T]
    h2t = h2t.reshape(B, 8, 128, T).transpose(0, 2, 1, 3)  # [B, 128, 8, T]
    h2t = h2t.astype(BF)

    wa_r = w_a.reshape(D, H, M, M + 1)
    wv_r = w_v.reshape(D, H, M)
    wo_r = w_out_proj.reshape(H, M, D)

    mask = np.zeros((128, 2), dtype=np.float32)
    for p in range(128):
        w = (p // 16) % 2
        mask[p, w] = 1.0

    in_maps = []
    for c in range(N_CORES):
        b, grp = divmod(c, 4)
        base = grp * HPC
        # gather per-partition (unit, i) indices for each group
        hh = np.empty((2, 128), dtype=np.int64)
        ii = np.empty((2, 128), dtype=np.int64)
        for g in range(2):
            for p, (u8, i) in enumerate(umap):
                hh[g, p] = base + 8 * g + u8
                ii[g, p] = i
        # wa: [2, 17, 128cols, D] -> [2, 17, 128, 8, 128]
        wa_s = np.empty((2, 17, D, 128), dtype=np.float32)
        for g in range(2):
            for tau in range(17):
                wa_s[g, tau] = wa_r[:, hh[g], ii[g], tau]
        # [2, 17, D, 128c] -> [2, 17, 128dp, 8dc, 128c]
        wa_s = wa_s.reshape(2, 17, 8, 128, 128).transpose(0, 1, 3, 2, 4).astype(BF)
        # wv: [D, 128cols, 2] -> [128p, 8dc, 2, 128]
        wv_s = np.empty((D, 2, 128), dtype=np.float32)
        for g in range(2):
            wv_s[:, g, :] = wv_r[:, hh[g], ii[g]]
        wv_s = wv_s.reshape(8, 128, 2, 128).transpose(1, 0, 2, 3).astype(BF)
        # wo: [128p, 2, D]
        wo_s = np.empty((128, 2, D), dtype=np.float32)
        for g in range(2):
            wo_s[:, g, :] = wo_r[hh[g], ii[g], :]
        wo_s = wo_s.astype(BF)

        in_maps.append({
            "h2t": np.ascontiguousarray(h2t[b]),
            "wa": np.ascontiguousarray(wa_s),
            "wv": np.ascontiguousarray(wv_s),
            "wo": np.ascontiguousarray(wo_s),
            "mask": mask,
        })
    return in_maps


def lru_combine(x_after, results):
    """Sum per-core deltas into the residual. results: list of {'delta': ...}."""
    out = x_after.copy()
    for c in range(N_CORES):
        b = c // 4
        d = results[c]["delta"]  # [128, 8, T] f32, feature-major
        d = d.transpose(1, 0, 2).reshape(D, T)  # [D, T]
        out[b] += d.T
    return out




# ---------------------------------------------------------------------------
# attention launch (part 1)
# ---------------------------------------------------------------------------

def build_attn_nc(finalize=True):
    """Per-core: 4 heads of one batch. Emits partial delta (feature-major)."""
    nc = bacc.Bacc("TRN2", target_bir_lowering=False, debug=False,
                   enable_asserts=False, num_devices=N_CORES)
    h1_d = nc.dram_tensor("h1t", [128, 8, T], BF16, kind="ExternalInput").ap()
    wq_d = nc.dram_tensor("wq", [128, 8, 2, 128], BF16, kind="ExternalInput").ap()
    wqs_d = nc.dram_tensor("wqs", [128, 8, 2, 128], BF16, kind="ExternalInput").ap()
    wk_d = nc.dram_tensor("wk", [128, 8, 2, 128], BF16, kind="ExternalInput").ap()
    wks_d = nc.dram_tensor("wks", [128, 8, 2, 128], BF16, kind="ExternalInput").ap()
    wvv_d = nc.dram_tensor("wvv", [128, 8, 260], BF16, kind="ExternalInput").ap()
    wo_d = nc.dram_tensor("woa", [128, 2, D], BF16, kind="ExternalInput").ap()
    cos_d = nc.dram_tensor("cosT", [128, T], BF16, kind="ExternalInput").ap()
    sin_d = nc.dram_tensor("sinS", [128, T], BF16, kind="ExternalInput").ap()
    stair_d = nc.dram_tensor("stair", [128, 128], BF16, kind="ExternalInput").ap()
    out_d = nc.dram_tensor("delta", [128, 8, T], F32, kind="ExternalOutput").ap()

    ACT_COPY = mybir.ActivationFunctionType.Copy

    with tile.TileContext(nc) as tc:
        with (
            tc.tile_pool(name="big", bufs=1) as big,
            tc.tile_pool(name="etile", bufs=6) as epool,
            tc.tile_pool(name="outp", bufs=3) as outp,
            tc.tile_pool(name="psA", bufs=1, space="PSUM") as psA,
            tc.tile_pool(name="psB", bufs=2, space="PSUM") as psB,
        ):
            h1_sb = big.tile([128, 8, T], BF16)
            wq_sb = big.tile([128, 8, 2, 128], BF16)
            wqs_sb = big.tile([128, 8, 2, 128], BF16)
            wk_sb = big.tile([128, 8, 2, 128], BF16)
            wks_sb = big.tile([128, 8, 2, 128], BF16)
            wvv_sb = big.tile([128, 8, 260], BF16)
            wo_sb = big.tile([128, 2, D], BF16)
            cos_sb = big.tile([128, T], BF16)
            sin_sb = big.tile([128, T], BF16)
            stair_sb = big.tile([128, 128], BF16)
            ones_sb = big.tile([1, 64], F32)
            q_sb = big.tile([128, 2, T], BF16)
            k_sb = big.tile([128, 2, T], BF16)
            v_sb = big.tile([128, 8, 260], BF16)
            o_sb = big.tile([128, 2, T], BF16)
            t1 = big.tile([128, TH], F32)
            t2 = big.tile([128, TH], F32)
            rinv = big.tile([1, TH], F32)
            rb = big.tile([64, TH], BF16)

            nc.sync.dma_start(h1_sb[:], h1_d[:])
            nc.sync.dma_start(wq_sb[:], wq_d[:])
            nc.sync.dma_start(wqs_sb[:], wqs_d[:])
            nc.sync.dma_start(wk_sb[:], wk_d[:])
            nc.sync.dma_start(wks_sb[:], wks_d[:])
            nc.sync.dma_start(wvv_sb[:], wvv_d[:])
            nc.sync.dma_start(wo_sb[:], wo_d[:])
            nc.sync.dma_start(cos_sb[:], cos_d[:])
            nc.sync.dma_start(sin_sb[:], sin_d[:])
            nc.sync.dma_start(stair_sb[:], stair_d[:])
            nc.vector.memset(ones_sb[:], 1.0)

            # ---- q/k with rope ----
            for dst, w_sb, ws_sb in ((q_sb, wq_sb, wqs_sb), (k_sb, wk_sb, wks_sb)):
                for m2 in range(2):
                    for th in range(2):
                        pq = psA.tile([128, TH], F32, name="pq")
                        pqs = psA.tile([128, TH], F32, name="pqs")
                        for dc in range(8):
                            nc.tensor.matmul(
                                pq[:], w_sb[:, dc, m2], h1_sb[:, dc, bass.ts(th, TH)],
                                start=(dc == 0), stop=(dc == 7))
                        for dc in range(8):
                            nc.tensor.matmul(
                                pqs[:], ws_sb[:, dc, m2], h1_sb[:, dc, bass.ts(th, TH)],
                                start=(dc == 0), stop=(dc == 7))
                        nc.vector.tensor_tensor(
                            t1[:], pq[:], cos_sb[:, bass.ts(th, TH)], MULT)
                        nc.vector.tensor_tensor(
                            t2[:], pqs[:], sin_sb[:, bass.ts(th, TH)], MULT)
                        nc.vector.tensor_tensor(
                            dst[:, m2, bass.ts(th, TH)], t1[:], t2[:], ADD)

            # ---- v token-major (with ones cols) ----
            for tc8 in range(8):
                pv = psA.tile([128, 260], F32, name="pva")
                for dc in range(8):
                    nc.tensor.matmul(
                        pv[:], h1_sb[:, dc, bass.ts(tc8, 128)], wvv_sb[:, dc],
                        start=(dc == 0), stop=(dc == 7))
                nc.scalar.activation(v_sb[:, tc8], pv[:], ACT_COPY)
            for hcol in range(4):
                nc.vector.memset(v_sb[:, :, 65 * hcol + 64:65 * hcol + 65], 1.0)

            # ---- attention per (head, query-chunk) ----
            for h in range(4):
                m2, r0 = h // 2, 64 * (h % 2)
                rows = slice(r0, r0 + 64)
                for qc in range(2):
                    po = psA.tile([128, TH], F32, name="po")[:65]
                    n_kc = 4 * qc + 4
                    for kc in range(n_kc):
                        ps = psB.tile([128, TH], F32, name="ps")
                        nc.tensor.matmul(
                            ps[:], k_sb[rows, m2, bass.ts(kc, 128)],
                            q_sb[rows, m2, bass.ts(qc, TH)],
                            start=True, stop=True)
                        et = epool.tile([128, TH], BF16, name="et")
                        nc.scalar.activation(et[:], ps[:], EXP)
                        sub = kc - 4 * qc
                        if sub >= 0:
                            if sub > 0:
                                nc.vector.memset(et[:, 0:128 * sub], 0.0)
                            nc.vector.tensor_tensor(
                                et[:, bass.ts(sub, 128)], et[:, bass.ts(sub, 128)],
                                stair_sb[:], MULT)
                        nc.tensor.matmul(
                            po[:], v_sb[:, kc, 65 * h:65 * h + 65], et[:],
                            start=(kc == 0), stop=(kc == n_kc - 1))
                    # normalize: o = po[0:64] * (1/po[64]) broadcast
                    nc.vector.reciprocal(rinv[:], po[64:65, :])
                    pb = psA.tile([64, TH], F32, name="pb", tag="pq")
                    nc.tensor.matmul(pb[:], ones_sb[:], rinv[:],
                                     start=True, stop=True)
                    nc.scalar.activation(rb[:], pb[:], ACT_COPY)
                    nc.vector.tensor_tensor(
                        o_sb[rows, m2, bass.ts(qc, TH)], po[0:64, :], rb[:], MULT)

            # ---- delta = wo.T @ o ----
            for m in range(8):
                for th in range(2):
                    pd = psB.tile([128, TH], F32, name="pd", tag="ps")
                    for fc in range(2):
                        nc.tensor.matmul(
                            pd[:], wo_sb[:, fc, bass.ts(m, 128)],
                            o_sb[:, fc, bass.ts(th, TH)],
                            start=(fc == 0), stop=(fc == 1))
                    ot = outp.tile([128, TH], F32, name="ota")
                    nc.scalar.copy(ot[:], pd[:])
                    nc.sync.dma_start(out_d[:, m, bass.ts(th, TH)], ot[:])

    if finalize:
        nc.finalize()
        nc.m = get_hw_module(nc.m)
    return nc


def attn_shards(h1, w_qkv, w_attn_out):
    """Per-core inputs for the attention launch. h1 = rmsnorm(x)*norm_w."""
    h1t = np.ascontiguousarray(h1.transpose(0, 2, 1))            # [B, D, T]
    h1t = h1t.reshape(B, 8, 128, T).transpose(0, 2, 1, 3).astype(BF)

    wq_full = w_qkv[:, 0 * D:1 * D].reshape(D, NH, HD)
    wk_full = w_qkv[:, 1 * D:2 * D].reshape(D, NH, HD)
    wv_full = w_qkv[:, 2 * D:3 * D].reshape(D, NH, HD)
    wo_full = w_attn_out.reshape(NH, HD, D)

    scale = float(HD) ** -0.25
    inv_freq = 1.0 / (ROPE_BASE ** (np.arange(0, HD, 2, dtype=np.float32) / HD))
    freqs = np.arange(T, dtype=np.float32)[:, None] * inv_freq[None, :]
    emb = np.concatenate([freqs, freqs], axis=-1)                # [T, 64]
    cos = np.cos(emb).T                                          # [64, T]
    sin = np.sin(emb).T
    cosT = np.concatenate([cos, cos], axis=0).astype(BF)         # [128, T]
    sinS_h = sin.copy()
    sinS_h[0:32] = -sin[0:32]
    sinS = np.concatenate([sinS_h, sinS_h], axis=0).astype(BF)

    stair = (np.arange(128)[None, :] * 0 + np.arange(128)[None, :]
             >= np.arange(128)[:, None]).astype(np.float32).astype(BF)

    def pack_w(wsub):
        # wsub [D, 2, 64] (two head slots) -> [128dp, 8dc, 128cols]
        w = wsub.reshape(D, 128)
        return w.reshape(8, 128, 128).transpose(1, 0, 2)

    in_maps = []
    for c in range(N_CORES):
        b, grp = divmod(c, 4)
        heads = [4 * grp + i for i in range(4)]
        wq_c = np.empty((128, 8, 2, 128), dtype=np.float32)
        wqs_c = np.empty_like(wq_c)
        wk_c = np.empty_like(wq_c)
        wks_c = np.empty_like(wq_c)
        for m2 in range(2):
            hs = heads[2 * m2:2 * m2 + 2]
            wq_sub = wq_full[:, hs, :] * scale                  # [D, 2, 64]
            wk_sub = wk_full[:, hs, :] * scale
            swap = np.concatenate(
                [np.arange(32, 64), np.arange(0, 32)])          # j -> (j+32)%64
            wq_c[:, :, m2, :] = pack_w(wq_sub)
            wqs_c[:, :, m2, :] = pack_w(wq_sub[:, :, swap])
            wk_c[:, :, m2, :] = pack_w(wk_sub)
            wks_c[:, :, m2, :] = pack_w(wk_sub[:, :, swap])
        wvv = np.zeros((D, 260), dtype=np.float32)
        for i, hh in enumerate(heads):
            wvv[:, 65 * i:65 * i + 64] = wv_full[:, hh, :]
        wvv_c = wvv.reshape(8, 128, 260).transpose(1, 0, 2)
        wo_c = np.empty((128, 2, D), dtype=np.float32)
        for fc in range(2):
            hs = heads[2 * fc:2 * fc + 2]
            wo_c[:, fc, :] = wo_full[hs].reshape(128, D)
        in_maps.append({
            "h1t": np.ascontiguousarray(h1t[b]),
            "wq": wq_c.astype(BF), "wqs": wqs_c.astype(BF),
            "wk": wk_c.astype(BF), "wks": wks_c.astype(BF),
            "wvv": np.ascontiguousarray(wvv_c).astype(BF),
            "woa": wo_c.astype(BF),
            "cosT": cosT, "sinS": sinS, "stair": stair,
        })
    return in_maps


def attn_combine(x, results):
    out = x.copy()
    for c in range(N_CORES):
        b = c // 4
        d = results[c]["delta"].transpose(1, 0, 2).reshape(D, T)
        out[b] += d.T
    return out


# ---------------------------------------------------------------------------
# attention (host reference)
# ---------------------------------------------------------------------------

def _host_attention(x, attn_norm_w, w_qkv, w_attn_out):
    h = _rmsnorm(x, attn_norm_w)
    qkv = (h.reshape(B * T, D) @ w_qkv).reshape(B, T, 3, NH, HD)
    q, k, v = qkv[:, :, 0], qkv[:, :, 1], qkv[:, :, 2]
    inv_freq = 1.0 / (ROPE_BASE ** (np.arange(0, HD, 2, dtype=np.float32) / HD))
    freqs = np.arange(T, dtype=np.float32)[:, None] * inv_freq[None, :]
    emb = np.concatenate([freqs, freqs], axis=-1).astype(np.float32)
    cos = np.cos(emb)[None, :, None, :]
    sin = np.sin(emb)[None, :, None, :]

    def rope(t):
        t1, t2 = np.split(t, 2, axis=-1)
        return t * cos + np.concatenate([-t2, t1], axis=-1) * sin

    q, k = rope(q), rope(k)
    qh = q.transpose(0, 2, 1, 3)
    kh = k.transpose(0, 2, 1, 3)
    vh = v.transpose(0, 2, 1, 3)
    scores = np.matmul(qh, kh.transpose(0, 1, 3, 2)) / np.float32(np.sqrt(HD))
    causal = np.tril(np.ones((T, T), dtype=bool))
    scores = np.where(causal[None, None], scores, np.float32(-1e30))
    m = scores.max(axis=-1, keepdims=True)
    e = np.exp(scores - m)
    attn = e / e.sum(axis=-1, keepdims=True)
    o = np.matmul(attn, vh).transpose(0, 2, 1, 3).reshape(B, T, D)
    return x + (o.reshape(B * T, D) @ w_attn_out).reshape(B, T, D)


# ---------------------------------------------------------------------------
# entry point
# ---------------------------------------------------------------------------

LAST_EXEC_NS = 0
TRACE = bool(int(__import__("os").environ.get("BENCH_TRACE", "0")))


def _run(nc, in_maps):
    global LAST_EXEC_NS
    res = bass_utils.run_bass_kernel_spmd(
        nc, in_maps, list(range(N_CORES)), trace=TRACE)
    if res.exec_time_ns:
        LAST_EXEC_NS += res.exec_time_ns
    return res


def kernel(x, attn_norm_w, w_qkv, w_attn_out, lru_norm_w, w_v, w_a, w_out_proj):
    global LAST_EXEC_NS
    LAST_EXEC_NS = 0
    x = np.asarray(x, dtype=np.float32)
    attn_norm_w = np.asarray(attn_norm_w, dtype=np.float32)
    w_qkv = np.asarray(w_qkv, dtype=np.float32)
    w_attn_out = np.asarray(w_attn_out, dtype=np.float32)
    lru_norm_w = np.asarray(lru_norm_w, dtype=np.float32)
    w_v = np.asarray(w_v, dtype=np.float32)
    w_a = np.asarray(w_a, dtype=np.float32)
    w_out_proj = np.asarray(w_out_proj, dtype=np.float32)

    # ---- launch 1: attention ----
    h1 = _rmsnorm(x, attn_norm_w)
    if "attn" not in _cache:
        _cache["attn"] = build_attn_nc()
    res1 = _run(_cache["attn"], attn_shards(h1, w_qkv, w_attn_out))
    x_after = attn_combine(x, res1.results)

    # ---- launch 2: LRU ----
    h2 = _rmsnorm(x_after, lru_norm_w)
    if "lru" not in _cache:
        _cache["lru"] = build_lru_nc()
    res2 = _run(_cache["lru"], lru_shards(h2, w_v, w_a, w_out_proj))
    out = lru_combine(x_after, res2.results)
    return out.astype(np.float32)
